# revision 3
# baseline (speedup 1.0000x reference)
"""Trainium2 Bass kernel for grouped-expert 3-layer MLP (MoE) — expert-parallel.

Sharding: expert-parallel across 8 NeuronCores (E=16 -> 2 experts/core, full
batch B=8192 per core). Host pre-transposes each core's x-shard to
feature-major [2, DIN, B] (pure layout marshalling), device computes partial
sums over its 2 experts as out_c[DOUT, B]; host unshards by summing the 8
partials and transposing.

Per-core pipeline (all f32r, feature-major activations, nb=512 batch tiles,
processed in PAIRS for layer-3 column-tiling):
  h1T[hb]  = relu(W1[:,hb].T @ xT + b1)          4 blocks of [128, nb]
  h2T[gb]  = relu(sum_hb W2[hb,gb].T @ h1T + b2)
  L3: two batch tiles run concurrently on PE column-groups (0,0)/(0,64):
      po[0:64]   += sum_{e,gb} W3[gb].T @ h2T(bt even)
      po[64:128] += sum_{e,gb} W3[gb].T @ h2T(bt odd)
  epilogue: ACT adds summed b3 during PSUM->SBUF evacuation, DMA to
  out[DOUT, B] feature-major.

No PE transposes, no collectives; weights loaded once and SBUF-resident.
"""

import os
from contextlib import ExitStack

import bass_rust
import numpy as np

import concourse.bass as bass
import concourse.tile as tile
from concourse import bacc, mybir
from concourse.bass_utils import run_bass_kernel_spmd
from concourse.masks import make_identity

E_TOT, DIN, H, DOUT = 16, 128, 512, 64
B_FULL = 8192
N_CORES = 8
E_LOC = E_TOT // N_CORES  # 2 experts per core
HB = H // 128  # 4 h-blocks
F32 = mybir.dt.float32
FR = mybir.dt.float32r
BF = mybir.dt.bfloat16


def build_nc(nb=512):
    B = B_FULL
    nbt = B // nb  # 16 batch tiles
    assert nbt % 2 == 0
    nc = bacc.Bacc("TRN2", target_bir_lowering=False, debug=False)

    xt = nc.dram_tensor("xt", [E_LOC, DIN, B], F32, kind="ExternalInput")
    W1 = nc.dram_tensor("W1", [E_LOC, DIN, H], F32, kind="ExternalInput")
    b1 = nc.dram_tensor("b1", [E_LOC, H], F32, kind="ExternalInput")
    W2 = nc.dram_tensor("W2", [E_LOC, H, H], F32, kind="ExternalInput")
    b2 = nc.dram_tensor("b2", [E_LOC, H], F32, kind="ExternalInput")
    W3 = nc.dram_tensor("W3", [E_LOC, H, DOUT], F32, kind="ExternalInput")
    b3 = nc.dram_tensor("b3", [E_LOC, DOUT], F32, kind="ExternalInput")
    out = nc.dram_tensor("out", [DOUT, B], F32, kind="ExternalOutput")

    RELU = mybir.ActivationFunctionType.Relu
    IDENT = mybir.ActivationFunctionType.Identity
    ADD = mybir.AluOpType.add
    MAX = mybir.AluOpType.max

    with tile.TileContext(nc) as tc, ExitStack() as ctx:
        consts = ctx.enter_context(tc.tile_pool(name="consts", bufs=1))
        w1p = ctx.enter_context(tc.tile_pool(name="w1p", bufs=1))
        w2p = ctx.enter_context(tc.tile_pool(name="w2p", bufs=1))
        w3p = ctx.enter_context(tc.tile_pool(name="w3p", bufs=1))
        xtp = ctx.enter_context(tc.tile_pool(name="xtp", bufs=4))
        h1p = ctx.enter_context(tc.tile_pool(name="h1p", bufs=6))
        h2p = ctx.enter_context(tc.tile_pool(name="h2p", bufs=6))
        obp = ctx.enter_context(tc.tile_pool(name="obp", bufs=3))
        p1p = ctx.enter_context(tc.tile_pool(name="p1p", bufs=4, space="PSUM"))
        p2p = ctx.enter_context(tc.tile_pool(name="p2p", bufs=3, space="PSUM"))
        pop = ctx.enter_context(tc.tile_pool(name="pop", bufs=1, space="PSUM"))

        ident = consts.tile([128, 128], F32)
        make_identity(nc, ident)
        # warm the PE's HAM clock gate with dummy matmuls while weights load
        wrm = consts.tile([128, 512], F32)
        nc.gpsimd.memset(wrm, 1.0)
        wrmr = consts.tile([128, 512], BF)
        nc.vector.tensor_copy(wrmr, wrm)
        pwu = pop.tile([128, 512], F32, tag="po", name="pwu")
        for i in range(12):
            nc.tensor.matmul(pwu, wrmr[:, 0:128], wrmr, start=True, stop=True)

        # ---- biases ----
        # b1/b2 natural [E_LOC, H] -> per-partition layout via PE transpose:
        # bXs[p, hb*E_LOC + e] = bX[e, hb*128 + p]
        b1n = consts.tile([E_LOC, H], F32)
        nc.gpsimd.dma_start(out=b1n, in_=b1[:, :])
        b2n = consts.tile([E_LOC, H], F32)
        nc.gpsimd.dma_start(out=b2n, in_=b2[:, :])
        b3n = consts.tile([E_LOC, DOUT], F32)
        nc.gpsimd.dma_start(out=b3n, in_=b3[:, :])
        b1s = consts.tile([128, HB * E_LOC], F32)
        b2s = consts.tile([128, HB * E_LOC], F32)
        for bn, bs in ((b1n, b1s), (b2n, b2s)):
            pb = p2p.tile([128, HB * E_LOC], F32, tag="p2", name="pb")
            for hb in range(HB):
                nc.tensor.transpose(
                    pb[:, hb * E_LOC : (hb + 1) * E_LOC],
                    bn[:, hb * 128 : (hb + 1) * 128],
                    ident[:E_LOC, :E_LOC],
                )
            nc.vector.tensor_copy(bs, pb)
        # b3sum[p] = sum_e b3[e, p] on partitions 0..63, replicated to 64..127
        pb3 = p2p.tile([DOUT, E_LOC], F32, tag="p2", name="pb3")
        nc.tensor.transpose(pb3, b3n, ident[:E_LOC, :E_LOC])
        b3s = consts.tile([DOUT, E_LOC], F32)
        nc.vector.tensor_copy(b3s, pb3)
        b3sum = consts.tile([128, 1], F32)
        nc.vector.reduce_sum(b3sum[0:DOUT, :], b3s, axis=bass_rust.AxisListType.X)
        # replicate to upper partition half via SBUF->SBUF DMA
        nc.sync.dma_start(out=b3sum[64:128, :], in_=b3sum[0:64, :])

        # ---- weights (resident, f32r bit-identical to f32) ----
        w1t = w1p.tile([DIN, E_LOC, H], BF)
        w2t = w2p.tile([128, E_LOC, HB, H], BF)
        w3t = w3p.tile([128, E_LOC, HB, DOUT], BF)
        # Early weights (W1s, W2e0): f32 via parallel HWDGE queues + engine cast
        # to bf16 (ACT/DVE are idle at startup). Later weights via gpsimd SWDGE.
        w2f = w2p.tile([128, HB, H], F32)
        w1f = w1p.tile([DIN, E_LOC, H], F32)
        for whb in range(HB):
            (nc.sync if whb % 2 == 0 else nc.scalar).dma_start(
                out=w2f[:, whb, :],
                in_=W2[0, whb * 128 : (whb + 1) * 128, :],
            )
        nc.sync.dma_start(out=w1f[:, 0, :], in_=W1[0])
        nc.scalar.dma_start(out=w1f[:, 1, :], in_=W1[1])
        for whb in range(HB):
            if whb % 2 == 0:
                nc.scalar.copy(w2t[:, 0, whb, :], w2f[:, whb, :])
            else:
                nc.vector.tensor_copy(w2t[:, 0, whb, :], w2f[:, whb, :])
        nc.scalar.copy(w1t[:, 0, :], w1f[:, 0, :])
        nc.vector.tensor_copy(w1t[:, 1, :], w1f[:, 1, :])
        for whb in range(HB):
            nc.gpsimd.dma_start(
                out=w2t[:, 1, whb, :],
                in_=W2[1, whb * 128 : (whb + 1) * 128, :],
            )
        for e in range(E_LOC):
            nc.gpsimd.dma_start(
                out=w3t[:, e, :, :],
                in_=W3[e].rearrange("(hb p) o -> p hb o", p=128),
            )

        # ---- main loop: L1 runs one batch-tile ahead of L2/L3 ----
        def emit_l1(bt, e):
            b0 = bt * nb
            xtt = xtp.tile([DIN, nb], BF, tag="xt")
            nc.gpsimd.dma_start(out=xtt, in_=xt[e, :, b0 : b0 + nb])
            h1 = h1p.tile([128, HB, nb], BF, tag="h1")
            for hb in range(HB):
                ps = p1p.tile([128, nb], F32, tag="p1")
                nc.tensor.matmul(
                    ps,
                    w1t[:, e, hb * 128 : (hb + 1) * 128],
                    xtt,
                    start=True,
                    stop=True,
                )
                bias = b1s[:, hb * E_LOC + e : hb * E_LOC + e + 1]
                if hb in (0, 3):
                    nc.scalar.activation(h1[:, hb, :], ps, RELU, bias=bias)
                else:
                    nc.vector.tensor_scalar(h1[:, hb, :], ps, bias, 0.0, ADD, MAX)
            return h1

        def emit_l2(e, h1):
            h2 = h2p.tile([128, HB, nb], BF, tag="h2")
            for gb in range(HB):
                ps = p2p.tile([128, nb], F32, tag="p2")
                for hb in range(HB):
                    nc.tensor.matmul(
                        ps,
                        w2t[:, e, hb, gb * 128 : (gb + 1) * 128],
                        h1[:, hb, :],
                        start=(hb == 0),
                        stop=(hb == HB - 1),
                    )
                bias = b2s[:, gb * E_LOC + e : gb * E_LOC + e + 1]
                if gb in (0, 3):
                    nc.scalar.activation(h2[:, gb, :], ps, RELU, bias=bias)
                else:
                    nc.vector.tensor_scalar(h2[:, gb, :], ps, bias, 0.0, ADD, MAX)
            return h2

        h1s = {e: emit_l1(0, e) for e in range(E_LOC)}
        pwu2 = pop.tile([128, nb], F32, tag="po", name="pwu2")
        for i in range(20):
            nc.tensor.matmul(pwu2, wrmr[:, 0:128], wrmr, start=True, stop=True)
        for bt in range(nbt):
            b0 = bt * nb
            h2s = {}
            h1cur = h1s
            h1s = {}
            h2s[0] = emit_l2(0, h1cur[0])
            if bt + 1 < nbt:
                h1s[0] = emit_l1(bt + 1, 0)
                h1s[1] = emit_l1(bt + 1, 1)
            h2s[1] = emit_l2(1, h1cur[1])

            # ---- layer 3: accumulate over experts and gb in PSUM ----
            po = pop.tile([DOUT, nb], F32, tag="po")
            idx = 0
            for e in range(E_LOC):
                for gb in range(HB):
                    idx += 1
                    nc.tensor.matmul(
                        po,
                        w3t[:, e, gb, :],
                        h2s[e][:, gb, :],
                        start=(idx == 1),
                        stop=(idx == E_LOC * HB),
                    )

            # ---- epilogue: bias add + store (feature-major) ----
            ob = obp.tile([DOUT, nb], F32, tag="ob")
            hn = nb // 2
            nc.scalar.activation(ob[:, 0:hn], po[:, 0:hn], IDENT, bias=b3sum[0:DOUT, :])
            nc.vector.tensor_scalar_add(ob[:, hn:nb], po[:, hn:nb], b3sum[0:DOUT, :])
            nc.sync.dma_start(out=out[:, b0 : b0 + hn], in_=ob[:, 0:hn])
            nc.scalar.dma_start(out=out[:, b0 + hn : b0 + nb], in_=ob[:, hn:nb])

    nc.compile()
    return nc


_NC_CACHE = {}


def _get_nc():
    if "nc" not in _NC_CACHE:
        _NC_CACHE["nc"] = build_nc()
    return _NC_CACHE["nc"]


def kernel(x, W1, b1, W2, b2, W3, b3):
    x = np.asarray(x, dtype=np.float32)
    W1 = np.asarray(W1, dtype=np.float32)
    b1 = np.asarray(b1, dtype=np.float32)
    W2 = np.asarray(W2, dtype=np.float32)
    b2 = np.asarray(b2, dtype=np.float32)
    W3 = np.asarray(W3, dtype=np.float32)
    b3 = np.asarray(b3, dtype=np.float32)

    nc = _get_nc()
    in_maps = []
    for c in range(N_CORES):
        sl = slice(E_LOC * c, E_LOC * (c + 1))
        in_maps.append(
            {
                "xt": np.ascontiguousarray(x[:, sl, :].transpose(1, 2, 0)),
                "W1": np.ascontiguousarray(W1[sl]),
                "b1": np.ascontiguousarray(b1[sl]),
                "W2": np.ascontiguousarray(W2[sl]),
                "b2": np.ascontiguousarray(b2[sl]),
                "W3": np.ascontiguousarray(W3[sl]),
                "b3": np.ascontiguousarray(b3[sl]),
            }
        )
    trace = bool(int(os.environ.get("KERNEL_TRACE", "0")))
    kwargs = {}
    if trace and os.environ.get("KERNEL_TRACE_DIR"):
        kwargs["tmpdir"] = os.environ["KERNEL_TRACE_DIR"]
    res = run_bass_kernel_spmd(nc, in_maps, list(range(N_CORES)), trace=trace, **kwargs)
    if trace:
        kernel.last_results = res
    acc = res.results[0]["out"].astype(np.float64)
    for c in range(1, N_CORES):
        acc += res.results[c]["out"]
    return np.ascontiguousarray(acc.T.astype(np.float32))


# revision 4
# speedup vs baseline: 1.0043x; 1.0043x over previous
"""Trainium2 Bass kernel for grouped-expert 3-layer MLP (MoE) — expert-parallel.

Sharding: expert-parallel across 8 NeuronCores (E=16 -> 2 experts/core, full
batch B=8192 per core). Host pre-transposes each core's x-shard to
feature-major [2, DIN, B] (pure layout marshalling), device computes partial
sums over its 2 experts as out_c[DOUT, B]; host unshards by summing the 8
partials and transposing.

Per-core pipeline (all f32r, feature-major activations, nb=512 batch tiles,
processed in PAIRS for layer-3 column-tiling):
  h1T[hb]  = relu(W1[:,hb].T @ xT + b1)          4 blocks of [128, nb]
  h2T[gb]  = relu(sum_hb W2[hb,gb].T @ h1T + b2)
  L3: two batch tiles run concurrently on PE column-groups (0,0)/(0,64):
      po[0:64]   += sum_{e,gb} W3[gb].T @ h2T(bt even)
      po[64:128] += sum_{e,gb} W3[gb].T @ h2T(bt odd)
  epilogue: ACT adds summed b3 during PSUM->SBUF evacuation, DMA to
  out[DOUT, B] feature-major.

No PE transposes, no collectives; weights loaded once and SBUF-resident.
"""

import os
from contextlib import ExitStack

import bass_rust
import numpy as np

import concourse.bass as bass
import concourse.tile as tile
from concourse import bacc, mybir
from concourse.bass_utils import run_bass_kernel_spmd
from concourse.masks import make_identity

E_TOT, DIN, H, DOUT = 16, 128, 512, 64
B_FULL = 8192
N_CORES = 8
E_LOC = E_TOT // N_CORES  # 2 experts per core
HB = H // 128  # 4 h-blocks
F32 = mybir.dt.float32
FR = mybir.dt.float32r
BF = mybir.dt.bfloat16


def build_nc(nb=512):
    B = B_FULL
    nbt = B // nb  # 16 batch tiles
    assert nbt % 2 == 0
    nc = bacc.Bacc("TRN2", target_bir_lowering=False, debug=False)

    xt = nc.dram_tensor("xt", [E_LOC, DIN, B], F32, kind="ExternalInput")
    W1 = nc.dram_tensor("W1", [E_LOC, DIN, H], F32, kind="ExternalInput")
    b1 = nc.dram_tensor("b1", [E_LOC, H], F32, kind="ExternalInput")
    W2 = nc.dram_tensor("W2", [E_LOC, H, H], F32, kind="ExternalInput")
    b2 = nc.dram_tensor("b2", [E_LOC, H], F32, kind="ExternalInput")
    W3 = nc.dram_tensor("W3", [E_LOC, H, DOUT], F32, kind="ExternalInput")
    b3 = nc.dram_tensor("b3", [E_LOC, DOUT], F32, kind="ExternalInput")
    out = nc.dram_tensor("out", [DOUT, B], F32, kind="ExternalOutput")

    RELU = mybir.ActivationFunctionType.Relu
    IDENT = mybir.ActivationFunctionType.Identity
    ADD = mybir.AluOpType.add
    MAX = mybir.AluOpType.max

    with tile.TileContext(nc) as tc, ExitStack() as ctx:
        consts = ctx.enter_context(tc.tile_pool(name="consts", bufs=1))
        w1p = ctx.enter_context(tc.tile_pool(name="w1p", bufs=1))
        w2p = ctx.enter_context(tc.tile_pool(name="w2p", bufs=1))
        w3p = ctx.enter_context(tc.tile_pool(name="w3p", bufs=1))
        xtp = ctx.enter_context(tc.tile_pool(name="xtp", bufs=4))
        h1p = ctx.enter_context(tc.tile_pool(name="h1p", bufs=6))
        h2p = ctx.enter_context(tc.tile_pool(name="h2p", bufs=6))
        obp = ctx.enter_context(tc.tile_pool(name="obp", bufs=3))
        p1p = ctx.enter_context(tc.tile_pool(name="p1p", bufs=4, space="PSUM"))
        p2p = ctx.enter_context(tc.tile_pool(name="p2p", bufs=3, space="PSUM"))
        pop = ctx.enter_context(tc.tile_pool(name="pop", bufs=1, space="PSUM"))

        ident = consts.tile([128, 128], F32)
        make_identity(nc, ident)
        # warm the PE's HAM clock gate with dummy matmuls while weights load
        wrm = consts.tile([128, 512], F32)
        nc.gpsimd.memset(wrm, 1.0)
        wrmr = consts.tile([128, 512], BF)
        nc.vector.tensor_copy(wrmr, wrm)
        pwu = pop.tile([128, 512], F32, tag="po", name="pwu")
        for i in range(12):
            nc.tensor.matmul(pwu, wrmr[:, 0:128], wrmr, start=True, stop=True)

        # ---- biases ----
        # b1/b2 natural [E_LOC, H] -> per-partition layout via PE transpose:
        # bXs[p, hb*E_LOC + e] = bX[e, hb*128 + p]
        b1n = consts.tile([E_LOC, H], F32)
        nc.gpsimd.dma_start(out=b1n, in_=b1[:, :])
        b2n = consts.tile([E_LOC, H], F32)
        nc.gpsimd.dma_start(out=b2n, in_=b2[:, :])
        b3n = consts.tile([E_LOC, DOUT], F32)
        nc.gpsimd.dma_start(out=b3n, in_=b3[:, :])
        b1s = consts.tile([128, HB * E_LOC], F32)
        b2s = consts.tile([128, HB * E_LOC], F32)
        for bn, bs in ((b1n, b1s), (b2n, b2s)):
            pb = p2p.tile([128, HB * E_LOC], F32, tag="p2", name="pb")
            for hb in range(HB):
                nc.tensor.transpose(
                    pb[:, hb * E_LOC : (hb + 1) * E_LOC],
                    bn[:, hb * 128 : (hb + 1) * 128],
                    ident[:E_LOC, :E_LOC],
                )
            nc.vector.tensor_copy(bs, pb)
        # b3sum[p] = sum_e b3[e, p] on partitions 0..63, replicated to 64..127
        pb3 = p2p.tile([DOUT, E_LOC], F32, tag="p2", name="pb3")
        nc.tensor.transpose(pb3, b3n, ident[:E_LOC, :E_LOC])
        b3s = consts.tile([DOUT, E_LOC], F32)
        nc.vector.tensor_copy(b3s, pb3)
        b3sum = consts.tile([128, 1], F32)
        nc.vector.reduce_sum(b3sum[0:DOUT, :], b3s, axis=bass_rust.AxisListType.X)
        # replicate to upper partition half via SBUF->SBUF DMA
        nc.sync.dma_start(out=b3sum[64:128, :], in_=b3sum[0:64, :])

        # ---- weights (resident, f32r bit-identical to f32) ----
        w1t = w1p.tile([DIN, E_LOC, H], BF)
        w2t = w2p.tile([128, E_LOC, HB, H], BF)
        w3t = w3p.tile([128, E_LOC, HB, DOUT], BF)
        # Early weights (W1s, W2e0): f32 via parallel HWDGE queues + engine cast
        # to bf16 (ACT/DVE are idle at startup). Later weights via gpsimd SWDGE.
        w2f = w2p.tile([128, HB, H], F32)
        w1f = w1p.tile([DIN, E_LOC, H], F32)
        for whb in range(HB):
            (nc.sync if whb % 2 == 0 else nc.scalar).dma_start(
                out=w2f[:, whb, :],
                in_=W2[0, whb * 128 : (whb + 1) * 128, :],
            )
        nc.sync.dma_start(out=w1f[:, 0, :], in_=W1[0])
        nc.scalar.dma_start(out=w1f[:, 1, :], in_=W1[1])
        for whb in range(HB):
            if whb % 2 == 0:
                nc.scalar.copy(w2t[:, 0, whb, :], w2f[:, whb, :])
            else:
                nc.vector.tensor_copy(w2t[:, 0, whb, :], w2f[:, whb, :])
        nc.scalar.copy(w1t[:, 0, :], w1f[:, 0, :])
        nc.vector.tensor_copy(w1t[:, 1, :], w1f[:, 1, :])
        for whb in range(HB):
            nc.gpsimd.dma_start(
                out=w2t[:, 1, whb, :],
                in_=W2[1, whb * 128 : (whb + 1) * 128, :],
            )
        for e in range(E_LOC):
            nc.gpsimd.dma_start(
                out=w3t[:, e, :, :],
                in_=W3[e].rearrange("(hb p) o -> p hb o", p=128),
            )

        # ---- main loop: L1 runs one batch-tile ahead of L2/L3 ----
        def emit_l1(bt, e):
            b0 = bt * nb
            xtt = xtp.tile([DIN, nb], BF, tag="xt")
            nc.gpsimd.dma_start(out=xtt, in_=xt[e, :, b0 : b0 + nb])
            h1 = h1p.tile([128, HB, nb], BF, tag="h1")
            for hb in range(HB):
                ps = p1p.tile([128, nb], F32, tag="p1")
                nc.tensor.matmul(
                    ps,
                    w1t[:, e, hb * 128 : (hb + 1) * 128],
                    xtt,
                    start=True,
                    stop=True,
                )
                bias = b1s[:, hb * E_LOC + e : hb * E_LOC + e + 1]
                if hb in (0, 3):
                    nc.scalar.activation(h1[:, hb, :], ps, RELU, bias=bias)
                else:
                    nc.vector.tensor_scalar(h1[:, hb, :], ps, bias, 0.0, ADD, MAX)
            return h1

        def emit_l2(e, h1):
            h2 = h2p.tile([128, HB, nb], BF, tag="h2")
            for gb in range(HB):
                ps = p2p.tile([128, nb], F32, tag="p2")
                for hb in range(HB):
                    nc.tensor.matmul(
                        ps,
                        w2t[:, e, hb, gb * 128 : (gb + 1) * 128],
                        h1[:, hb, :],
                        start=(hb == 0),
                        stop=(hb == HB - 1),
                    )
                bias = b2s[:, gb * E_LOC + e : gb * E_LOC + e + 1]
                if gb in (0, 3):
                    nc.scalar.activation(h2[:, gb, :], ps, RELU, bias=bias)
                else:
                    nc.vector.tensor_scalar(h2[:, gb, :], ps, bias, 0.0, ADD, MAX)
            return h2

        h1s = {e: emit_l1(0, e) for e in range(E_LOC)}
        pwu2 = pop.tile([128, nb], F32, tag="po", name="pwu2")
        for i in range(20):
            nc.tensor.matmul(pwu2, wrmr[:, 0:128], wrmr, start=True, stop=True)
        for bt in range(nbt):
            b0 = bt * nb
            h2s = {}
            h1cur = h1s
            h1s = {}
            h2s[0] = emit_l2(0, h1cur[0])
            if bt + 1 < nbt:
                h1s[0] = emit_l1(bt + 1, 0)
            h2s[1] = emit_l2(1, h1cur[1])
            if bt + 1 < nbt:
                h1s[1] = emit_l1(bt + 1, 1)

            # ---- layer 3: accumulate over experts and gb in PSUM ----
            po = pop.tile([DOUT, nb], F32, tag="po")
            idx = 0
            for e in range(E_LOC):
                for gb in range(HB):
                    idx += 1
                    nc.tensor.matmul(
                        po,
                        w3t[:, e, gb, :],
                        h2s[e][:, gb, :],
                        start=(idx == 1),
                        stop=(idx == E_LOC * HB),
                    )

            # ---- epilogue: bias add + store (feature-major) ----
            ob = obp.tile([DOUT, nb], F32, tag="ob")
            hn = nb // 2
            nc.scalar.activation(ob[:, 0:hn], po[:, 0:hn], IDENT, bias=b3sum[0:DOUT, :])
            nc.vector.tensor_scalar_add(ob[:, hn:nb], po[:, hn:nb], b3sum[0:DOUT, :])
            nc.sync.dma_start(out=out[:, b0 : b0 + hn], in_=ob[:, 0:hn])
            nc.scalar.dma_start(out=out[:, b0 + hn : b0 + nb], in_=ob[:, hn:nb])

    nc.compile()
    return nc


_NC_CACHE = {}


def _get_nc():
    if "nc" not in _NC_CACHE:
        _NC_CACHE["nc"] = build_nc()
    return _NC_CACHE["nc"]


def kernel(x, W1, b1, W2, b2, W3, b3):
    x = np.asarray(x, dtype=np.float32)
    W1 = np.asarray(W1, dtype=np.float32)
    b1 = np.asarray(b1, dtype=np.float32)
    W2 = np.asarray(W2, dtype=np.float32)
    b2 = np.asarray(b2, dtype=np.float32)
    W3 = np.asarray(W3, dtype=np.float32)
    b3 = np.asarray(b3, dtype=np.float32)

    nc = _get_nc()
    in_maps = []
    for c in range(N_CORES):
        sl = slice(E_LOC * c, E_LOC * (c + 1))
        in_maps.append(
            {
                "xt": np.ascontiguousarray(x[:, sl, :].transpose(1, 2, 0)),
                "W1": np.ascontiguousarray(W1[sl]),
                "b1": np.ascontiguousarray(b1[sl]),
                "W2": np.ascontiguousarray(W2[sl]),
                "b2": np.ascontiguousarray(b2[sl]),
                "W3": np.ascontiguousarray(W3[sl]),
                "b3": np.ascontiguousarray(b3[sl]),
            }
        )
    trace = bool(int(os.environ.get("KERNEL_TRACE", "0")))
    kwargs = {}
    if trace and os.environ.get("KERNEL_TRACE_DIR"):
        kwargs["tmpdir"] = os.environ["KERNEL_TRACE_DIR"]
    res = run_bass_kernel_spmd(nc, in_maps, list(range(N_CORES)), trace=trace, **kwargs)
    if trace:
        kernel.last_results = res
    acc = res.results[0]["out"].astype(np.float64)
    for c in range(1, N_CORES):
        acc += res.results[c]["out"]
    return np.ascontiguousarray(acc.T.astype(np.float32))


# revision 5
# speedup vs baseline: 1.0043x; 1.0001x over previous
"""Trainium2 Bass kernel for grouped-expert 3-layer MLP (MoE) — expert-parallel.

Sharding: expert-parallel across 8 NeuronCores (E=16 -> 2 experts/core, full
batch B=8192 per core). Host pre-transposes each core's x-shard to
feature-major [2, DIN, B] (pure layout marshalling), device computes partial
sums over its 2 experts as out_c[DOUT, B]; host unshards by summing the 8
partials and transposing.

Per-core pipeline (all f32r, feature-major activations, nb=512 batch tiles,
processed in PAIRS for layer-3 column-tiling):
  h1T[hb]  = relu(W1[:,hb].T @ xT + b1)          4 blocks of [128, nb]
  h2T[gb]  = relu(sum_hb W2[hb,gb].T @ h1T + b2)
  L3: two batch tiles run concurrently on PE column-groups (0,0)/(0,64):
      po[0:64]   += sum_{e,gb} W3[gb].T @ h2T(bt even)
      po[64:128] += sum_{e,gb} W3[gb].T @ h2T(bt odd)
  epilogue: ACT adds summed b3 during PSUM->SBUF evacuation, DMA to
  out[DOUT, B] feature-major.

No PE transposes, no collectives; weights loaded once and SBUF-resident.
"""

import os
from contextlib import ExitStack

import bass_rust
import numpy as np

import concourse.bass as bass
import concourse.tile as tile
from concourse import bacc, mybir
from concourse.bass_utils import run_bass_kernel_spmd
from concourse.masks import make_identity

E_TOT, DIN, H, DOUT = 16, 128, 512, 64
B_FULL = 8192
N_CORES = 8
E_LOC = E_TOT // N_CORES  # 2 experts per core
HB = H // 128  # 4 h-blocks
F32 = mybir.dt.float32
FR = mybir.dt.float32r
BF = mybir.dt.bfloat16


def build_nc(nb=512):
    B = B_FULL
    nbt = B // nb  # 16 batch tiles
    assert nbt % 2 == 0
    nc = bacc.Bacc("TRN2", target_bir_lowering=False, debug=False)

    xt = nc.dram_tensor("xt", [E_LOC, DIN, B], F32, kind="ExternalInput")
    W1 = nc.dram_tensor("W1", [E_LOC, DIN, H], F32, kind="ExternalInput")
    b1 = nc.dram_tensor("b1", [E_LOC, H], F32, kind="ExternalInput")
    W2 = nc.dram_tensor("W2", [E_LOC, H, H], F32, kind="ExternalInput")
    b2 = nc.dram_tensor("b2", [E_LOC, H], F32, kind="ExternalInput")
    W3 = nc.dram_tensor("W3", [E_LOC, H, DOUT], F32, kind="ExternalInput")
    b3 = nc.dram_tensor("b3", [E_LOC, DOUT], F32, kind="ExternalInput")
    out = nc.dram_tensor("out", [DOUT, B], F32, kind="ExternalOutput")

    RELU = mybir.ActivationFunctionType.Relu
    IDENT = mybir.ActivationFunctionType.Identity
    ADD = mybir.AluOpType.add
    MAX = mybir.AluOpType.max

    with tile.TileContext(nc) as tc, ExitStack() as ctx:
        consts = ctx.enter_context(tc.tile_pool(name="consts", bufs=1))
        w1p = ctx.enter_context(tc.tile_pool(name="w1p", bufs=1))
        w2p = ctx.enter_context(tc.tile_pool(name="w2p", bufs=1))
        w3p = ctx.enter_context(tc.tile_pool(name="w3p", bufs=1))
        xtp = ctx.enter_context(tc.tile_pool(name="xtp", bufs=4))
        h1p = ctx.enter_context(tc.tile_pool(name="h1p", bufs=6))
        h2p = ctx.enter_context(tc.tile_pool(name="h2p", bufs=6))
        obp = ctx.enter_context(tc.tile_pool(name="obp", bufs=3))
        p1p = ctx.enter_context(tc.tile_pool(name="p1p", bufs=4, space="PSUM"))
        p2p = ctx.enter_context(tc.tile_pool(name="p2p", bufs=3, space="PSUM"))
        pop = ctx.enter_context(tc.tile_pool(name="pop", bufs=1, space="PSUM"))

        ident = consts.tile([128, 128], F32)
        make_identity(nc, ident)
        # warm the PE's HAM clock gate with dummy matmuls while weights load
        wrm = consts.tile([128, 512], F32)
        nc.gpsimd.memset(wrm, 1.0)
        wrmr = consts.tile([128, 512], BF)
        nc.vector.tensor_copy(wrmr, wrm)
        pwu = pop.tile([128, 512], F32, tag="po", name="pwu")
        for i in range(12):
            nc.tensor.matmul(pwu, wrmr[:, 0:128], wrmr, start=True, stop=True)

        # ---- biases ----
        # b1/b2 natural [E_LOC, H] -> per-partition layout via PE transpose:
        # bXs[p, hb*E_LOC + e] = bX[e, hb*128 + p]
        b1n = consts.tile([E_LOC, H], F32)
        nc.gpsimd.dma_start(out=b1n, in_=b1[:, :])
        b2n = consts.tile([E_LOC, H], F32)
        nc.gpsimd.dma_start(out=b2n, in_=b2[:, :])
        b3n = consts.tile([E_LOC, DOUT], F32)
        nc.gpsimd.dma_start(out=b3n, in_=b3[:, :])
        b1s = consts.tile([128, HB * E_LOC], F32)
        b2s = consts.tile([128, HB * E_LOC], F32)
        for bn, bs in ((b1n, b1s), (b2n, b2s)):
            pb = p2p.tile([128, HB * E_LOC], F32, tag="p2", name="pb")
            for hb in range(HB):
                nc.tensor.transpose(
                    pb[:, hb * E_LOC : (hb + 1) * E_LOC],
                    bn[:, hb * 128 : (hb + 1) * 128],
                    ident[:E_LOC, :E_LOC],
                )
            nc.vector.tensor_copy(bs, pb)
        # b3sum[p] = sum_e b3[e, p] on partitions 0..63, replicated to 64..127
        pb3 = p2p.tile([DOUT, E_LOC], F32, tag="p2", name="pb3")
        nc.tensor.transpose(pb3, b3n, ident[:E_LOC, :E_LOC])
        b3s = consts.tile([DOUT, E_LOC], F32)
        nc.vector.tensor_copy(b3s, pb3)
        b3sum = consts.tile([128, 1], F32)
        nc.vector.reduce_sum(b3sum[0:DOUT, :], b3s, axis=bass_rust.AxisListType.X)
        # replicate to upper partition half via SBUF->SBUF DMA
        nc.sync.dma_start(out=b3sum[64:128, :], in_=b3sum[0:64, :])

        # ---- weights (resident, f32r bit-identical to f32) ----
        w1t = w1p.tile([DIN, E_LOC, H], BF)
        w2t = w2p.tile([128, E_LOC, HB, H], BF)
        w3t = w3p.tile([128, E_LOC, HB, DOUT], BF)
        # Early weights (W1s, W2e0): f32 via parallel HWDGE queues + engine cast
        # to bf16 (ACT/DVE are idle at startup). Later weights via gpsimd SWDGE.
        w2f = w2p.tile([128, HB, H], F32)
        w1f = w1p.tile([DIN, E_LOC, H], F32)
        for whb in range(HB):
            (nc.sync if whb % 2 == 0 else nc.scalar).dma_start(
                out=w2f[:, whb, :],
                in_=W2[0, whb * 128 : (whb + 1) * 128, :],
            )
        nc.sync.dma_start(out=w1f[:, 0, :], in_=W1[0])
        nc.scalar.dma_start(out=w1f[:, 1, :], in_=W1[1])
        for whb in range(HB):
            if whb % 2 == 0:
                nc.scalar.copy(w2t[:, 0, whb, :], w2f[:, whb, :])
            else:
                nc.vector.tensor_copy(w2t[:, 0, whb, :], w2f[:, whb, :])
        nc.scalar.copy(w1t[:, 0, :], w1f[:, 0, :])
        nc.vector.tensor_copy(w1t[:, 1, :], w1f[:, 1, :])
        for whb in range(HB):
            nc.gpsimd.dma_start(
                out=w2t[:, 1, whb, :],
                in_=W2[1, whb * 128 : (whb + 1) * 128, :],
            )
        for e in range(E_LOC):
            nc.gpsimd.dma_start(
                out=w3t[:, e, :, :],
                in_=W3[e].rearrange("(hb p) o -> p hb o", p=128),
            )

        # ---- main loop: L1 runs one batch-tile ahead of L2/L3 ----
        def emit_l1(bt, e):
            b0 = bt * nb
            xtt = xtp.tile([DIN, nb], BF, tag="xt")
            nc.gpsimd.dma_start(out=xtt, in_=xt[e, :, b0 : b0 + nb])
            h1 = h1p.tile([128, HB, nb], BF, tag="h1")
            for hb in range(HB):
                ps = p1p.tile([128, nb], F32, tag="p1")
                nc.tensor.matmul(
                    ps,
                    w1t[:, e, hb * 128 : (hb + 1) * 128],
                    xtt,
                    start=True,
                    stop=True,
                )
                bias = b1s[:, hb * E_LOC + e : hb * E_LOC + e + 1]
                if hb in (0, 3):
                    nc.scalar.activation(h1[:, hb, :], ps, RELU, bias=bias)
                else:
                    nc.vector.tensor_scalar(h1[:, hb, :], ps, bias, 0.0, ADD, MAX)
            return h1

        def emit_l2(e, h1):
            h2 = h2p.tile([128, HB, nb], BF, tag="h2")
            for gb in range(HB):
                ps = p2p.tile([128, nb], F32, tag="p2")
                for hb in range(HB):
                    nc.tensor.matmul(
                        ps,
                        w2t[:, e, hb, gb * 128 : (gb + 1) * 128],
                        h1[:, hb, :],
                        start=(hb == 0),
                        stop=(hb == HB - 1),
                    )
                bias = b2s[:, gb * E_LOC + e : gb * E_LOC + e + 1]
                if gb in (0, 3):
                    nc.scalar.activation(h2[:, gb, :], ps, RELU, bias=bias)
                else:
                    nc.vector.tensor_scalar(h2[:, gb, :], ps, bias, 0.0, ADD, MAX)
            return h2

        h1s = {e: emit_l1(0, e) for e in range(E_LOC)}
        pwu2 = pop.tile([128, nb], F32, tag="po", name="pwu2")
        for i in range(20):
            nc.tensor.matmul(pwu2, wrmr[:, 0:128], wrmr, start=True, stop=True)
        for bt in range(nbt):
            b0 = bt * nb
            h2s = {}
            h1cur = h1s
            h1s = {}
            h2s[0] = emit_l2(0, h1cur[0])
            if bt + 1 < nbt:
                h1s[0] = emit_l1(bt + 1, 0)
                h1s[1] = emit_l1(bt + 1, 1)
            h2s[1] = emit_l2(1, h1cur[1])

            # ---- layer 3: accumulate over experts and gb in PSUM ----
            po = pop.tile([DOUT, nb], F32, tag="po")
            idx = 0
            for e in range(E_LOC):
                for gb in range(HB):
                    idx += 1
                    nc.tensor.matmul(
                        po,
                        w3t[:, e, gb, :],
                        h2s[e][:, gb, :],
                        start=(idx == 1),
                        stop=(idx == E_LOC * HB),
                    )

            # ---- epilogue: bias add + store (feature-major) ----
            ob = obp.tile([DOUT, nb], F32, tag="ob")
            hn = nb // 2
            nc.scalar.activation(ob[:, 0:hn], po[:, 0:hn], IDENT, bias=b3sum[0:DOUT, :])
            nc.vector.tensor_scalar_add(ob[:, hn:nb], po[:, hn:nb], b3sum[0:DOUT, :])
            nc.sync.dma_start(out=out[:, b0 : b0 + hn], in_=ob[:, 0:hn])
            nc.scalar.dma_start(out=out[:, b0 + hn : b0 + nb], in_=ob[:, hn:nb])

    nc.compile()
    return nc


_NC_CACHE = {}


def _get_nc():
    if "nc" not in _NC_CACHE:
        _NC_CACHE["nc"] = build_nc()
    return _NC_CACHE["nc"]


def kernel(x, W1, b1, W2, b2, W3, b3):
    x = np.asarray(x, dtype=np.float32)
    W1 = np.asarray(W1, dtype=np.float32)
    b1 = np.asarray(b1, dtype=np.float32)
    W2 = np.asarray(W2, dtype=np.float32)
    b2 = np.asarray(b2, dtype=np.float32)
    W3 = np.asarray(W3, dtype=np.float32)
    b3 = np.asarray(b3, dtype=np.float32)

    nc = _get_nc()
    in_maps = []
    for c in range(N_CORES):
        sl = slice(E_LOC * c, E_LOC * (c + 1))
        in_maps.append(
            {
                "xt": np.ascontiguousarray(x[:, sl, :].transpose(1, 2, 0)),
                "W1": np.ascontiguousarray(W1[sl]),
                "b1": np.ascontiguousarray(b1[sl]),
                "W2": np.ascontiguousarray(W2[sl]),
                "b2": np.ascontiguousarray(b2[sl]),
                "W3": np.ascontiguousarray(W3[sl]),
                "b3": np.ascontiguousarray(b3[sl]),
            }
        )
    trace = bool(int(os.environ.get("KERNEL_TRACE", "0")))
    kwargs = {}
    if trace and os.environ.get("KERNEL_TRACE_DIR"):
        kwargs["tmpdir"] = os.environ["KERNEL_TRACE_DIR"]
    res = run_bass_kernel_spmd(nc, in_maps, list(range(N_CORES)), trace=trace, **kwargs)
    if trace:
        kernel.last_results = res
    acc = res.results[0]["out"].astype(np.float64)
    for c in range(1, N_CORES):
        acc += res.results[c]["out"]
    return np.ascontiguousarray(acc.T.astype(np.float32))
